# revision 1
# baseline (speedup 1.0000x reference)
"""Trainium2 Bass kernel for nn_CrossAttention (B=2, S=2048, E=1024, H=16, d=64).

Sharding: 8 cores = 2 batches x 4 query-blocks of 512 rows. Each core gets its
query block + the full values[b] for its batch; no collectives needed.

Per-core device pipeline (feature-major / transposed layouts throughout):
  1. PE-transpose bf16 inputs -> v_inT [d-major], q_inT.
  2. Pair-packed projections with host-folded weights:
       kT = (Wk@Wv) @ v_inT   (bkv dropped: per-row softmax-invariant shift)
       qT = Wq @ q_inT + bq
       v0 = v_inT.T @ blkdiag(WvT)  (natural layout, bv folded into bd on host)
  3. scores.T = kT.T-slices @ qT  (row-tiled 2-head concurrent matmuls)
  4. E = exp(scores.T * 0.125) on ACT -> bf16
  5. U.T = [v0 | ones].T @ E  (ones column yields softmax denominator as row 64)
  6. out.T = U.T * (1/r) via PE-replicated reciprocal rows
  7. final = out.T-slices @ WdT + (bd + Wd@tile(bv))  -> natural layout -> DMA
"""

import sys

for _p in ("/opt/trn_rl_repo",):
    if _p not in sys.path:
        sys.path.insert(0, _p)

from contextlib import ExitStack

import ml_dtypes
import numpy as np

import concourse.bass as bass
import concourse.tile as tile
from concourse import bacc, mybir
from concourse.bass_utils import run_bass_kernel_spmd
from concourse.masks import make_identity

F32 = mybir.dt.float32
BF16 = mybir.dt.bfloat16
F32R = mybir.dt.float32r
EXP = mybir.ActivationFunctionType.Exp

B, S, E, H, D = 2, 2048, 1024, 16, 64
N_CORES = 8
SQB = S * B // N_CORES  # 512 query rows per core
NP_BF16 = ml_dtypes.bfloat16

_CACHE = {}


def _build_program():
    nc = bacc.Bacc("TRN2", target_bir_lowering=False, debug=False, num_devices=N_CORES)

    # inputs are shipped pre-transposed (feature-major) — host-side marshaling
    qT_in = nc.dram_tensor("qT_in", [E, SQB], BF16, kind="ExternalInput").ap()
    vT_in = nc.dram_tensor("vT_in", [E, S], BF16, kind="ExternalInput").ap()
    wkv2 = nc.dram_tensor("wkv2", [128, 128], BF16, kind="ExternalInput").ap()
    wq2 = nc.dram_tensor("wq2", [128, 128], BF16, kind="ExternalInput").ap()
    wv2 = nc.dram_tensor("wv2", [128, 128], BF16, kind="ExternalInput").ap()
    bq2 = nc.dram_tensor("bq2", [128, 1], F32, kind="ExternalInput").ap()
    sel = nc.dram_tensor("sel", [16, 1024], F32R, kind="ExternalInput").ap()
    wdT = nc.dram_tensor("wdT", [E, E], BF16, kind="ExternalInput").ap()
    bd_rep = nc.dram_tensor("bd_rep", [128, E], F32, kind="ExternalInput").ap()
    out = nc.dram_tensor("out", [SQB, E], F32, kind="ExternalOutput").ap()

    with tile.TileContext(nc) as tc, ExitStack() as ctx:
        # ---- pools ----
        wpool = ctx.enter_context(tc.tile_pool(name="w", bufs=1))
        # natural input tiles (persistent) and E tiles (ring)
        natp = ctx.enter_context(tc.tile_pool(name="natp", bufs=1))
        ep = ctx.enter_context(tc.tile_pool(name="ep", bufs=20))
        # feature-major rings
        vtp = ctx.enter_context(tc.tile_pool(name="vtp", bufs=3))
        ktp = ctx.enter_context(tc.tile_pool(name="ktp", bufs=2))
        qtp = ctx.enter_context(tc.tile_pool(name="qtp", bufs=1))
        qintp = ctx.enter_context(tc.tile_pool(name="qintp", bufs=2))
        vnatp = ctx.enter_context(tc.tile_pool(name="vnat", bufs=3))
        u2p = ctx.enter_context(tc.tile_pool(name="u2", bufs=1))
        outp = ctx.enter_context(tc.tile_pool(name="outp", bufs=1))
        osbp = ctx.enter_context(tc.tile_pool(name="osb", bufs=2))
        misc_ps = ctx.enter_context(tc.tile_pool(name="mps", bufs=3, space="PSUM"))
        sc_ps = ctx.enter_context(tc.tile_pool(name="scps", bufs=2, space="PSUM"))
        u_ps = ctx.enter_context(tc.tile_pool(name="ups", bufs=1, space="PSUM"))

        # ---- constants ----
        wkv2_s = wpool.tile([128, 128], BF16, tag="wkv2")
        nc.sync.dma_start(wkv2_s[:], wkv2[:])
        wq2_s = wpool.tile([128, 128], BF16, tag="wq2")
        nc.sync.dma_start(wq2_s[:], wq2[:])
        wv2_s = wpool.tile([128, 128], BF16, tag="wv2")
        nc.sync.dma_start(wv2_s[:], wv2[:])
        bq2_s = wpool.tile([128, 1], F32, tag="bq2")
        nc.sync.dma_start(bq2_s[:], bq2[:])
        sel_s = wpool.tile([16, 1024], F32R, tag="sel")
        nc.sync.dma_start(sel_s[:], sel[:])
        bd_s = wpool.tile([128, E], F32, tag="bd")
        nc.sync.dma_start(bd_s[:], bd_rep[:])
        wd_s = []
        for kk in range(8):
            t = wpool.tile([128, E], BF16, tag=f"wd{kk}")
            nc.sync.dma_start(t[:], wdT[kk * 128 : (kk + 1) * 128, :])
            wd_s.append(t)

        # ---- per-pair: load feature-major inputs + projections + attention ----
        rgather = wpool.tile([16, SQB], F32, tag="rgather")
        U2 = [
            u2p.tile([128, SQB], F32, tag=f"u2_{p}", name=f"u2_{p}")
            for p in range(8)
        ]

        for p in range(8):
            # the pair's feature block: vinT [128, 2048], qinT [128, 512]
            vt = vtp.tile([128, S], BF16, tag="vinT", name=f"vinT{p}")
            nc.sync.dma_start(vt[:], vT_in[p * 128 : (p + 1) * 128, :])
            qt_in = qintp.tile([128, SQB], BF16, tag="qinT", name=f"qinT{p}")
            nc.sync.dma_start(qt_in[:], qT_in[p * 128 : (p + 1) * 128, :])
            vinT = {p: vt}
            qinT = {p: qt_in}

            # kT pair [128, 2048] bf16 (no bias: softmax-shift-invariant)
            kt = ktp.tile([128, S], BF16, tag="kT", name=f"kT{p}")
            for c in range(4):
                ps = misc_ps.tile([128, 512], F32, tag="mps")
                nc.tensor.matmul(
                    ps[:], wkv2_s[:], vinT[p][:, c * 512 : (c + 1) * 512],
                    start=True, stop=True,
                )
                nc.vector.tensor_copy(kt[:, c * 512 : (c + 1) * 512], ps[:])
            # qT pair [128, 512] bf16 (+bq)
            qt = qtp.tile([128, SQB], BF16, tag=f"qT{p}", name=f"qT{p}")
            ps = misc_ps.tile([128, 512], F32, tag="mps")
            nc.tensor.matmul(ps[:], wq2_s[:], qinT[p][:], start=True, stop=True)
            nc.vector.tensor_scalar_add(qt[:], ps[:], bq2_s[:])
            # v natural pair -> per-head [128, 16, 65] with ones col
            va = vnatp.tile([128, 16, 65], BF16, tag="vnat", name=f"vnat{2 * p}")
            vb = vnatp.tile([128, 16, 65], BF16, tag="vnat", name=f"vnat{2 * p + 1}")
            for g in range(4):
                ps = misc_ps.tile([128, 512], F32, tag="mps")
                for tt in range(4):
                    t = g * 4 + tt
                    nc.tensor.matmul(
                        ps[:, tt * 128 : (tt + 1) * 128],
                        vinT[p][:, t * 128 : (t + 1) * 128],
                        wv2_s[:],
                        start=True, stop=True,
                    )
                psv = ps.rearrange("p (t c) -> p t c", c=128)
                sl = slice(g * 4, (g + 1) * 4)
                nc.vector.tensor_copy(va[:, sl, 0:64], psv[:, :, 0:64])
                nc.vector.tensor_copy(vb[:, sl, 0:64], psv[:, :, 64:128])
            nc.gpsimd.memset(va[:, :, 64:65], 1.0)
            nc.gpsimd.memset(vb[:, :, 64:65], 1.0)

            # scores.T + exp, row-tiled 2 heads concurrently
            EA, EB = [], []
            for g in range(8):
                psA = sc_ps.tile([128, 1024], F32, tag="scps")
                psB = sc_ps.tile([128, 1024], F32, tag="scps")
                for tt in range(2):
                    t = g * 2 + tt
                    nc.tensor.matmul(
                        psA[:, tt * 512 : (tt + 1) * 512],
                        kt[0:64, t * 128 : (t + 1) * 128],
                        qt[0:64, :],
                        start=True, stop=True, tile_position=(0, 0),
                    )
                    nc.tensor.matmul(
                        psB[:, tt * 512 : (tt + 1) * 512],
                        kt[64:128, t * 128 : (t + 1) * 128],
                        qt[64:128, :],
                        start=True, stop=True, tile_position=(64, 0),
                    )
                ea = ep.tile([128, 1024], BF16, tag="E", name=f"ea{p}_{g}")
                nc.scalar.activation(ea[:], psA[:], EXP, scale=0.125)
                EA.append(ea)
                eb = ep.tile([128, 1024], BF16, tag="E", name=f"eb{p}_{g}")
                nc.scalar.activation(eb[:], psB[:], EXP, scale=0.125)
                EB.append(eb)

            # attn @ [v|1] per head; row 64 = softmax denominator
            for h2, (vv, EE) in enumerate(((0, EA), (1, EB))):
                vv, EE = (va, EA) if h2 == 0 else (vb, EB)
                h = 2 * p + h2
                ups = u_ps.tile([65, 512], F32, tag="ups")
                for t in range(16):
                    et = EE[t // 2][:, (t % 2) * 512 : (t % 2 + 1) * 512]
                    nc.tensor.matmul(
                        ups[:], vv[:, t, :], et,
                        start=(t == 0), stop=(t == 15),
                    )
                nc.vector.tensor_copy(
                    U2[p][h2 * 64 : (h2 + 1) * 64, :], ups[0:64, :]
                )
                # single-partition writes at arbitrary offsets are illegal for
                # compute engines (and DMA can't read PSUM): bounce the
                # denominator row through partition 0, then DMA into place
                rtmp = osbp.tile([1, SQB], F32, tag="rtmp", name=f"rtmp{h}")
                nc.vector.tensor_copy(rtmp[:], ups[64:65, :])
                nc.sync.dma_start(rgather[h : h + 1, :], rtmp[:])

        # ---- normalize ----
        rrec = wpool.tile([16, SQB], F32R, tag="rrec")
        with nc.allow_low_precision(reason="f32r is full fp32 range; f22 mantissa ok for softmax denom"):
            nc.vector.reciprocal(rrec[:], rgather[:])
        outT = []
        for p in range(8):
            rps = misc_ps.tile([128, 512], F32, tag="mps")
            nc.tensor.matmul(
                rps[:],
                sel_s[:, p * 128 : (p + 1) * 128],
                rrec[:],
                start=True, stop=True,
            )
            ot = outp.tile([128, SQB], BF16, tag=f"outT{p}")
            nc.vector.tensor_mul(ot[:], U2[p][:], rps[:])
            outT.append(ot)

        # ---- output projection + bias -> natural layout -> DMA ----
        for m in range(4):
            osb = osbp.tile([128, E], F32, tag="osb")
            for n in range(2):
                ps = sc_ps.tile([128, 512], F32, tag="scps")
                for kk in range(8):
                    nc.tensor.matmul(
                        ps[:],
                        outT[kk][:, m * 128 : (m + 1) * 128],
                        wd_s[kk][:, n * 512 : (n + 1) * 512],
                        start=(kk == 0), stop=(kk == 7),
                    )
                nc.vector.tensor_add(
                    osb[:, n * 512 : (n + 1) * 512], ps[:],
                    bd_s[:, n * 512 : (n + 1) * 512],
                )
            nc.sync.dma_start(out[m * 128 : (m + 1) * 128, :], osb[:])

    nc.compile()
    return nc


def kernel(queries, values, heads, Wv, bv, Wk, bk, Wq, bq, Wd, bd, **_):
    queries = np.asarray(queries, np.float32)
    values = np.asarray(values, np.float32)
    Wv, bv = np.asarray(Wv, np.float32), np.asarray(bv, np.float32)
    Wk = np.asarray(Wk, np.float32)
    Wq, bq = np.asarray(Wq, np.float32), np.asarray(bq, np.float32)
    Wd, bd = np.asarray(Wd, np.float32), np.asarray(bd, np.float32)
    assert int(heads) == H and queries.shape == (B, S, E)

    if "nc" not in _CACHE:
        _CACHE["nc"] = _build_program()
    nc = _CACHE["nc"]

    def blk(A):
        Z = np.zeros_like(A)
        return np.block([[A, Z], [Z, A]]).astype(NP_BF16)

    Wkv = Wk @ Wv
    wkv2 = blk(Wkv.T)
    wq2 = blk(Wq.T)
    wv2 = blk(Wv.T)
    bq2 = np.concatenate([bq, bq])[:, None].astype(np.float32)
    sel = np.zeros((16, 1024), np.float32)
    for p in range(8):
        for m in range(128):
            sel[2 * p + m // 64, p * 128 + m] = 1.0
    bv_full = np.tile(bv, H)
    bd_rep = np.tile((bd + Wd @ bv_full)[None, :], (128, 1)).astype(np.float32)
    wdT = np.ascontiguousarray(Wd.T).astype(NP_BF16)

    vT_b = [
        np.ascontiguousarray(values[b_].T).astype(NP_BF16) for b_ in range(B)
    ]
    common = dict(wkv2=wkv2, wq2=wq2, wv2=wv2, bq2=bq2, sel=sel, wdT=wdT,
                  bd_rep=bd_rep)
    in_maps = []
    for c in range(N_CORES):
        b_, qb = c // 4, c % 4
        in_maps.append(dict(
            qT_in=np.ascontiguousarray(
                queries[b_, qb * SQB : (qb + 1) * SQB, :].T
            ).astype(NP_BF16),
            vT_in=vT_b[b_],
            **common,
        ))

    _CACHE["last_in_maps"] = in_maps
    res = run_bass_kernel_spmd(nc, in_maps, list(range(N_CORES)))
    out = np.empty((B, S, E), np.float32)
    for c in range(N_CORES):
        b_, qb = c // 4, c % 4
        out[b_, qb * SQB : (qb + 1) * SQB, :] = res.results[c]["out"]
    return out



# revision 2
# speedup vs baseline: 1.0890x; 1.0890x over previous
"""Trainium2 Bass kernel for nn_CrossAttention (B=2, S=2048, E=1024, H=16, d=64).

Sharding: 8 cores = 2 batches x 4 query-blocks of 512 rows. Each core gets its
query block + the full values[b] for its batch; no collectives needed.

Algebra (host-folded): with q = q_in@Wq.T+bq, v = v_in@Wv.T+bv, k = v@Wk.T+bk:
  scores = q @ k.T = qe @ v_in.T + const(q-row)   [softmax-shift-invariant]
    where qe = q_in @ (Wq.T@Wk@Wv) + bq@Wk@Wv
  out    = attn @ v @ Wd.T + bd = (attn @ v_in) @ (Wd@blockdiag(Wv)).T
           + (bd + Wd@tile(bv))                   [attn rows sum to 1]
So the device never computes k or v projections.

Per-core device pipeline (feature-major / transposed layouts):
  1. qeT = WQ2 @ q_inT + cq2 per head-pair (WQ2 = blkdiag pair of fold)
  2. scores.T = v_inT.T-slices @ qeT   (row-tiled 2-head concurrent matmuls)
  3. E = exp(scores.T * 0.125): split between ACT (exact spline) and DVE
     (Schraudolph: bf16-bits = int16(x*A + B), ~3.3% max rel err)
  4. U.T = [v_in | ones].T @ E  (ones col yields softmax denom as row 64)
  5. out.T = U.T * (1/r) via PE-replicated reciprocal rows
  6. final = out.T-slices @ WdT' + bd'  -> natural layout -> DMA
"""

import sys

for _p in ("/opt/trn_rl_repo",):
    if _p not in sys.path:
        sys.path.insert(0, _p)

from contextlib import ExitStack

import ml_dtypes
import numpy as np

import concourse.bass as bass
import concourse.tile as tile
from concourse import bacc, mybir
from concourse.bass_utils import run_bass_kernel_spmd

F32 = mybir.dt.float32
BF16 = mybir.dt.bfloat16
F32R = mybir.dt.float32r
I16 = mybir.dt.int16
EXP = mybir.ActivationFunctionType.Exp
MULT = mybir.AluOpType.mult
ADD = mybir.AluOpType.add

B, S, E, H, D = 2, 2048, 1024, 16, 64
N_CORES = 8
SQB = S * B // N_CORES  # 512 query rows per core
NP_BF16 = ml_dtypes.bfloat16

# Schraudolph fast-exp constants: bf16bits(exp(x/8)) ~= int16(x*SCH_A + SCH_B)
LOG2E = 1.4426950408889634
SCH_A = 0.125 * 128.0 * LOG2E
SCH_B = 16256.0 - 5.5027  # HW rounds-to-nearest (probe-verified)

# per pair: psB tiles with g < ACT_B go to ACT, rest to DVE Schraudolph
ACT_B = 2

_CACHE = {}


def _build_program():
    nc = bacc.Bacc("TRN2", target_bir_lowering=False, debug=False, num_devices=N_CORES)

    qT_in = nc.dram_tensor("qT_in", [E, SQB], BF16, kind="ExternalInput").ap()
    vT_in = nc.dram_tensor("vT_in", [E, S], BF16, kind="ExternalInput").ap()
    vnat_in = nc.dram_tensor("vnat_in", [S, H * 65], BF16, kind="ExternalInput").ap()
    wq2 = nc.dram_tensor("wq2", [128, 128], BF16, kind="ExternalInput").ap()
    cq2 = nc.dram_tensor("cq2", [128, 1], F32, kind="ExternalInput").ap()
    sel = nc.dram_tensor("sel", [16, 1024], F32R, kind="ExternalInput").ap()
    wdT = nc.dram_tensor("wdT", [E, E], BF16, kind="ExternalInput").ap()
    bd_rep = nc.dram_tensor("bd_rep", [128, E], F32, kind="ExternalInput").ap()
    out = nc.dram_tensor("out", [SQB, E], F32, kind="ExternalOutput").ap()

    with tile.TileContext(nc) as tc, ExitStack() as ctx:
        # ---- pools ----
        wpool = ctx.enter_context(tc.tile_pool(name="w", bufs=1))
        ep = ctx.enter_context(tc.tile_pool(name="ep", bufs=18))
        u2p = ctx.enter_context(tc.tile_pool(name="u2", bufs=1))
        outp = ctx.enter_context(tc.tile_pool(name="outp", bufs=1))
        osbp = ctx.enter_context(tc.tile_pool(name="osb", bufs=2))
        sc_ps = ctx.enter_context(tc.tile_pool(name="scps", bufs=3, space="PSUM"))
        u_ps = ctx.enter_context(tc.tile_pool(name="ups", bufs=1, space="PSUM"))

        # ---- constants / persistent inputs ----
        wq2_s = wpool.tile([128, 128], BF16, tag="wq2")
        nc.sync.dma_start(wq2_s[:], wq2[:])
        cq2_s = wpool.tile([128, 1], F32, tag="cq2")
        nc.sync.dma_start(cq2_s[:], cq2[:])
        sel_s = wpool.tile([16, 1024], F32R, tag="sel")
        nc.sync.dma_start(sel_s[:], sel[:])
        bd_s = wpool.tile([128, E], F32, tag="bd")
        nc.sync.dma_start(bd_s[:], bd_rep[:])
        wd_s = []
        for kk in range(8):
            t = wpool.tile([128, E], BF16, tag=f"wd{kk}")
            nc.sync.dma_start(t[:], wdT[kk * 128 : (kk + 1) * 128, :])
            wd_s.append(t)
        # feature-major values, one persistent tile per head-pair
        vt = []
        for p in range(8):
            t = wpool.tile([128, S], BF16, tag=f"vt{p}")
            nc.sync.dma_start(t[:], vT_in[p * 128 : (p + 1) * 128, :])
            vt.append(t)
        # natural values + ones col, one tile per 128-row skv chunk
        vna = []
        for tch in range(16):
            t = wpool.tile([128, H * 65], BF16, tag=f"vna{tch}")
            nc.sync.dma_start(t[:], vnat_in[tch * 128 : (tch + 1) * 128, :])
            vna.append(t)
        # feature-major queries per pair
        qin = []
        for p in range(8):
            t = wpool.tile([128, SQB], BF16, tag=f"qin{p}")
            nc.sync.dma_start(t[:], qT_in[p * 128 : (p + 1) * 128, :])
            qin.append(t)

        # ---- qe projection: all pairs up-front (2 pairs per PSUM tile) ----
        qe_sb = wpool.tile([128, 8 * SQB], BF16, tag="qe")
        for pp in range(4):
            ps = sc_ps.tile([128, 1024], F32, tag="scps")
            for i in range(2):
                p = 2 * pp + i
                nc.tensor.matmul(
                    ps[:, i * 512 : (i + 1) * 512], wq2_s[:], qin[p][:],
                    start=True, stop=True,
                )
            nc.vector.tensor_scalar(
                qe_sb[:, pp * 1024 : (pp + 1) * 1024], ps[:], cq2_s[:], None,
                op0=ADD,
            )

        rgather = wpool.tile([16, SQB], F32, tag="rgather")
        U2 = [
            u2p.tile([128, SQB], BF16, tag=f"u2_{p}", name=f"u2_{p}")
            for p in range(8)
        ]

        # ---- per-pair: scores -> exp (ACT/DVE split) -> U ----
        for p in range(8):
            qe = qe_sb[:, p * SQB : (p + 1) * SQB]
            EA, EB = [], []
            for g in range(8):
                psA = sc_ps.tile([128, 1024], F32, tag="scps")
                psB = sc_ps.tile([128, 1024], F32, tag="scps")
                for tt in range(2):
                    t = g * 2 + tt
                    nc.tensor.matmul(
                        psA[:, tt * 512 : (tt + 1) * 512],
                        vt[p][0:64, t * 128 : (t + 1) * 128],
                        qe[0:64, :],
                        start=True, stop=True, tile_position=(0, 0),
                    )
                    nc.tensor.matmul(
                        psB[:, tt * 512 : (tt + 1) * 512],
                        vt[p][64:128, t * 128 : (t + 1) * 128],
                        qe[64:128, :],
                        start=True, stop=True, tile_position=(64, 0),
                    )
                ea = ep.tile([128, 1024], BF16, tag="E", name=f"ea{p}_{g}")
                nc.scalar.activation(ea[:], psA[:], EXP, scale=0.125)
                EA.append(ea)
                eb = ep.tile([128, 1024], BF16, tag="E", name=f"eb{p}_{g}")
                if g < ACT_B:
                    nc.scalar.activation(eb[:], psB[:], EXP, scale=0.125)
                else:
                    nc.vector.tensor_scalar(
                        eb[:].bitcast(I16), psB[:], SCH_A, SCH_B,
                        op0=MULT, op1=ADD,
                    )
                EB.append(eb)

            # attn @ [v_in|1] per head; row 64 = softmax denominator
            for h2 in range(2):
                EE = EA if h2 == 0 else EB
                h = 2 * p + h2
                ups = u_ps.tile([65, 512], F32, tag="ups")
                for t in range(16):
                    et = EE[t // 2][:, (t % 2) * 512 : (t % 2 + 1) * 512]
                    nc.tensor.matmul(
                        ups[:], vna[t][:, h * 65 : (h + 1) * 65], et,
                        start=(t == 0), stop=(t == 15),
                    )
                nc.vector.tensor_copy(
                    U2[p][h2 * 64 : (h2 + 1) * 64, :], ups[0:64, :]
                )
                # single-partition writes at arbitrary offsets are illegal for
                # compute engines (and DMA can't read PSUM): bounce the
                # denominator row through partition 0, then DMA into place
                rtmp = osbp.tile([1, SQB], F32, tag="rtmp", name=f"rtmp{h}")
                if h2 == 0:
                    nc.scalar.copy(rtmp[:], ups[64:65, :])
                else:
                    nc.vector.tensor_copy(rtmp[:], ups[64:65, :])
                nc.sync.dma_start(rgather[h : h + 1, :], rtmp[:])

        # ---- normalize ----
        rrec = wpool.tile([16, SQB], F32R, tag="rrec")
        with nc.allow_low_precision(reason="f32r full fp32 range; f22 mantissa ok for softmax denom"):
            nc.vector.reciprocal(rrec[:], rgather[:])
        outT = []
        for p in range(8):
            rps = u_ps.tile([128, 512], F32, tag="rps")
            nc.tensor.matmul(
                rps[:],
                sel_s[:, p * 128 : (p + 1) * 128],
                rrec[:],
                start=True, stop=True,
            )
            ot = outp.tile([128, SQB], BF16, tag=f"outT{p}")
            nc.vector.tensor_mul(ot[:], U2[p][:], rps[:])
            outT.append(ot)

        # ---- output projection + bias -> natural layout -> DMA ----
        for m in range(4):
            ps = sc_ps.tile([128, 1024], F32, tag="scps")
            for n in range(2):
                for kk in range(8):
                    nc.tensor.matmul(
                        ps[:, n * 512 : (n + 1) * 512],
                        outT[kk][:, m * 128 : (m + 1) * 128],
                        wd_s[kk][:, n * 512 : (n + 1) * 512],
                        start=(kk == 0), stop=(kk == 7),
                    )
            osb = osbp.tile([128, E], F32, tag="osb")
            nc.vector.tensor_add(osb[:], ps[:], bd_s[:])
            nc.sync.dma_start(out[m * 128 : (m + 1) * 128, :], osb[:])

    nc.compile()
    return nc


def kernel(queries, values, heads, Wv, bv, Wk, bk, Wq, bq, Wd, bd, **_):
    queries = np.asarray(queries, np.float32)
    values = np.asarray(values, np.float32)
    Wv, bv = np.asarray(Wv, np.float64), np.asarray(bv, np.float64)
    Wk = np.asarray(Wk, np.float64)
    Wq, bq = np.asarray(Wq, np.float64), np.asarray(bq, np.float64)
    Wd, bd = np.asarray(Wd, np.float64), np.asarray(bd, np.float64)
    assert int(heads) == H and queries.shape == (B, S, E)

    if "nc" not in _CACHE:
        _CACHE["nc"] = _build_program()
    nc = _CACHE["nc"]

    def blk(A):
        Z = np.zeros_like(A)
        return np.block([[A, Z], [Z, A]]).astype(NP_BF16)

    Wkv = Wk @ Wv
    wq2 = blk(Wq.T @ Wkv)                       # lhsT for qe projection
    cq2 = np.tile(Wkv.T @ bq, 2)[:, None].astype(np.float32)
    sel = np.zeros((16, 1024), np.float32)
    for p in range(8):
        for m in range(128):
            sel[2 * p + m // 64, p * 128 + m] = 1.0
    bv_full = np.tile(bv, H)
    bd_rep = np.tile((bd + Wd @ bv_full)[None, :], (128, 1)).astype(np.float32)
    Vblk = np.zeros((E, E))
    for h in range(H):
        Vblk[h * D : (h + 1) * D, h * D : (h + 1) * D] = Wv
    wdT = np.ascontiguousarray((Wd @ Vblk).T).astype(NP_BF16)

    vT_b, vnat_b = [], []
    for b_ in range(B):
        vT_b.append(np.ascontiguousarray(values[b_].T).astype(NP_BF16))
        vn = np.empty((S, H * 65), np.float32)
        vr = values[b_].reshape(S, H, D)
        for h in range(H):
            vn[:, h * 65 : h * 65 + 64] = vr[:, h, :]
            vn[:, h * 65 + 64] = 1.0
        vnat_b.append(vn.astype(NP_BF16))

    common = dict(wq2=wq2, cq2=cq2, sel=sel, wdT=wdT, bd_rep=bd_rep)
    in_maps = []
    for c in range(N_CORES):
        b_, qb = c // 4, c % 4
        in_maps.append(dict(
            qT_in=np.ascontiguousarray(
                queries[b_, qb * SQB : (qb + 1) * SQB, :].T
            ).astype(NP_BF16),
            vT_in=vT_b[b_],
            vnat_in=vnat_b[b_],
            **common,
        ))

    _CACHE["last_in_maps"] = in_maps
    res = run_bass_kernel_spmd(nc, in_maps, list(range(N_CORES)))
    out = np.empty((B, S, E), np.float32)
    for c in range(N_CORES):
        b_, qb = c // 4, c % 4
        out[b_, qb * SQB : (qb + 1) * SQB, :] = res.results[c]["out"]
    return out
